# revision 1
# baseline (speedup 1.0000x reference)
"""GQA (B=2,T=2048,C=2048, 32 Q heads / 8 KV heads, Dh=64) on 8 trn2 cores.

Sharding: core r -> batch b=r//4, rank=r%4 in its 4-core group.
Per core: 2 KV heads (8 Q heads), full 2048-token sequence of its batch.
Per-core partial output projection summed via in-group ReduceScatter over
tokens; host concatenates the 4 token shards per batch and adds bo.

Device pipeline (all matmuls fp32r, 1 cycle/row at N=512):
  P1  qT/kT/vT = Wqkv^T @ x^T (feature-major), bias fused on ScalarE
  P1b v_aug = transpose(vT) with a ones-column (softmax denominator trick)
  P2  per (kv j, token chunk): scoresT tile -> exp (ScalarE, scale=1/8)
      -> AV accumulate; row 0 of AV psum = softmax denominator
  P2b normalize YT by 1/denom (PE broadcast + DVE multiply)
  P3  out[t, c] = YT^T @ Wo_slice, DMA to DRAM partial
  P4  ReduceScatter(add) over 4-core group -> [512, 2048] token shard
"""

import sys
from contextlib import ExitStack

import numpy as np

sys.path.insert(0, "/opt/trn_rl_repo")

import concourse.bass as bass
import concourse.tile as tile
from concourse import bacc
from concourse import mybir
from concourse.bass_utils import run_bass_kernel_spmd

FP32 = mybir.dt.float32
FP32R = mybir.dt.float32r
AF = mybir.ActivationFunctionType

T = 2048
C = 2048
DH = 64
N_CORES = 8
GROUPS = [[0, 1, 2, 3], [4, 5, 6, 7]]


def _r(ap):
    return ap.bitcast(FP32R)


def _build_program():
    nc = bacc.Bacc(
        "TRN2", target_bir_lowering=False, debug=False, num_devices=N_CORES
    )
    xT = nc.dram_tensor("xT", [C, T], FP32, kind="ExternalInput").ap()
    wqkv = nc.dram_tensor("wqkv", [C, 768], FP32, kind="ExternalInput").ap()
    bqkv = nc.dram_tensor("bqkv", [128, 6], FP32, kind="ExternalInput").ap()
    wo = nc.dram_tensor("wo", [512, C], FP32, kind="ExternalInput").ap()
    sel_in = nc.dram_tensor("consts", [128, 384], FP32, kind="ExternalInput").ap()
    out_ext = nc.dram_tensor("out", [512, C], FP32, kind="ExternalOutput").ap()
    partial = nc.dram_tensor("partial", [T, C], FP32).ap()
    rs_out = nc.dram_tensor("rs_out", [512, C], FP32).ap()

    with tile.TileContext(nc) as tc:
        _emit(tc, xT, wqkv, bqkv, wo, sel_in, out_ext, partial, rs_out)
    nc.compile()
    return nc


def _emit(tc, xT, wqkv, bqkv, wo, sel_in, out_ext, partial, rs_out):
    nc = tc.nc
    NK = 16  # 128-row tiles of the contraction dim C
    NT = 4  # 512-token chunks

    with ExitStack() as top:
        pconst = top.enter_context(tc.tile_pool(name="const", bufs=1))
        pqkvT = top.enter_context(tc.tile_pool(name="qkvT", bufs=1))
        pvaug = top.enter_context(tc.tile_pool(name="vaug", bufs=1))

        ident = pconst.tile([128, 128], FP32R, tag="ident")
        nc.sync.dma_start(ident[:], sel_in[:, 0:128].bitcast(FP32R))
        bias_sb = pconst.tile([128, 6], FP32, tag="bias")
        nc.sync.dma_start(bias_sb[:], bqkv)
        # host-built selector row: [0:128] = lower-half indicator,
        # [128:256] = upper-half indicator (K=1 broadcast matmuls)
        sel1 = pconst.tile([1, 256], FP32, tag="sel1")
        nc.sync.dma_start(sel1[:], sel_in[0:1, 128:384])
        ones_sb = pconst.tile([128, 1], FP32R, tag="ones")
        nc.sync.dma_start(ones_sb[:], sel_in[:, 130:131].bitcast(FP32R))

        # persistent feature-major projections: q0..q3 | kT | vT
        qkvT = [
            pqkvT.tile([128, T], FP32R, tag=f"m{m}", name=f"qkvT{m}")
            if m != 4
            else None
            for m in range(6)
        ]
        # kT per kv head, the head's 64 dims duplicated in both partition
        # halves so scores matmuls can match q heads at base 0 or 64
        ktd = [pqkvT.tile([128, T], FP32R, tag=f"kt{j}", name=f"ktd{j}") for j in range(2)]
        # all 16 s-tiles of v_aug packed in one tile: block s = cols 130s..
        vaug = pvaug.tile([128, 130 * NK], FP32R, tag="vaug")

        # ---------------- Phase 1: projections ----------------
        with ExitStack() as ph1:
            pw = ph1.enter_context(tc.tile_pool(name="wq", bufs=1))
            px = ph1.enter_context(tc.tile_pool(name="x", bufs=20))
            p1 = ph1.enter_context(tc.tile_pool(name="p1", bufs=4, space="PSUM"))
            pt = ph1.enter_context(tc.tile_pool(name="ptr", bufs=2, space="PSUM"))

            w_sb = [pw.tile([128, 768], FP32R, tag=f"w{k}", name=f"wsb{k}") for k in range(NK)]
            for k in range(NK):
                nc.sync.dma_start(w_sb[k][:], wqkv[128 * k : 128 * (k + 1), :].bitcast(FP32R))

            for half in range(2):
                xs = []
                for k in range(NK):
                    xt = px.tile([128, 1024], FP32R, tag="x", name="xtile")
                    nc.sync.dma_start(
                        xt[:],
                        xT[128 * k : 128 * (k + 1), 1024 * half : 1024 * (half + 1)].bitcast(FP32R),
                    )
                    xs.append(xt)
                for m in range(6):
                    for t2 in range(2):
                        acc = p1.tile([128, 512], FP32, tag="acc", name="acc")
                        for k in range(NK):
                            nc.tensor.matmul(
                                acc[:],
                                _r(w_sb[k][:, 128 * m : 128 * (m + 1)]),
                                _r(xs[k][:, 512 * t2 : 512 * (t2 + 1)]),
                                start=(k == 0),
                                stop=(k == NK - 1),
                            )
                        tcol = half * 2 + t2
                        tsl = slice(512 * tcol, 512 * (tcol + 1))
                        if m == 4:
                            # kT: duplicate each kv head's 64 dims into both
                            # partition halves of its ktd tile
                            for j in range(2):
                                src = acc[64 * j : 64 * j + 64, :]
                                bia = bias_sb[64 * j : 64 * j + 64, m : m + 1]
                                nc.scalar.activation(
                                    ktd[j][0:64, tsl], src, AF.Identity, bias=bia
                                )
                                nc.scalar.activation(
                                    ktd[j][64:128, tsl], src, AF.Identity, bias=bia
                                )
                        else:
                            nc.scalar.activation(
                                qkvT[m][:, tsl],
                                acc[:],
                                AF.Identity,
                                bias=bias_sb[:, m : m + 1],
                            )

            # ---- Phase 1b: v_aug = [v_kv0 | 1 | v_kv1 | 1] token-major ----
            for s in range(NK):
                nc.vector.tensor_copy(
                    vaug[:, 130 * s + 64 : 130 * s + 65], ones_sb[:]
                )
                nc.vector.tensor_copy(
                    vaug[:, 130 * s + 129 : 130 * s + 130], ones_sb[:]
                )
            for s in range(NK):
                tr = pt.tile([128, 128], FP32R, tag="tr", name="tr")
                nc.tensor.transpose(
                    tr[:], qkvT[5][:, 128 * s : 128 * (s + 1)], ident[:]
                )
                o = 130 * s
                nc.vector.tensor_copy(vaug[:, o : o + 64], tr[:, 0:64])
                nc.vector.tensor_copy(vaug[:, o + 65 : o + 129], tr[:, 64:128])

        # ---------------- Phase 2: attention ----------------
        with ExitStack() as ph2:
            pYT = ph2.enter_context(tc.tile_pool(name="yt", bufs=1))
            pexp = ph2.enter_context(tc.tile_pool(name="exp", bufs=8))
            pwo = ph2.enter_context(tc.tile_pool(name="wo", bufs=1))
            pattn = ExitStack()
            ps = pattn.enter_context(tc.tile_pool(name="ps", bufs=3, space="PSUM"))
            pav = pattn.enter_context(tc.tile_pool(name="pav", bufs=4, space="PSUM"))
            pbc = pattn.enter_context(tc.tile_pool(name="pbc", bufs=1, space="PSUM"))
            pden = pattn.enter_context(tc.tile_pool(name="pden", bufs=8))

            YT = [pYT.tile([128, T], FP32R, tag=f"y{i}", name=f"YT{i}") for i in range(4)]
            wo_sb = [pwo.tile([128, C], FP32R, tag=f"wo{k}", name=f"wosb{k}") for k in range(4)]
            for k in range(4):
                nc.sync.dma_start(wo_sb[k][:], wo[128 * k : 128 * (k + 1), :].bitcast(FP32R))

            for j in range(2):  # local kv head
                for tck in range(NT):
                    tsl = slice(512 * tck, 512 * (tck + 1))
                    avs = [pav.tile([128, 512], FP32, tag="av", name="av") for _ in range(4)]
                    for s in range(NK):
                        for g in range(4):
                            h = 4 * j + g
                            qt = qkvT[h // 2]
                            po = 64 * (h % 2)
                            sp = ps.tile([128, 512], FP32, tag="sc", name="sc")
                            nc.tensor.matmul(
                                sp[:],
                                _r(ktd[j][po : po + 64, 128 * s : 128 * (s + 1)]),
                                _r(qt[po : po + 64, tsl]),
                                start=True,
                                stop=True,
                            )
                            et = pexp.tile([128, 512], FP32R, tag="exp", name="et")
                            nc.scalar.activation(et[:], sp[:], AF.Exp, scale=0.125)
                            nc.tensor.matmul(
                                avs[g][0:65, :],
                                _r(vaug[:, 130 * s + 65 * j : 130 * s + 65 * j + 65]),
                                _r(et[:]),
                                start=(s == 0),
                                stop=(s == NK - 1),
                            )
                    # finalize: copy Y rows, per-head reciprocal of the
                    # denominator row (psum row 64), broadcast + normalize
                    recips = []
                    for g in range(4):
                        h = 4 * j + g
                        po = 64 * (h % 2)
                        nc.vector.tensor_copy(
                            YT[h // 2][po : po + 64, tsl], avs[g][0:64, :]
                        )
                        rc = pden.tile([1, 512], FP32, tag="rc", name="rc")
                        nc.vector.reciprocal(rc[:], avs[g][64:65, :])
                        recips.append(rc)
                    for gp in range(2):
                        i = (4 * j + 2 * gp) // 2
                        bc = pbc.tile([128, 512], FP32, tag="bc", name="bc")
                        nc.tensor.matmul(
                            bc[:],
                            sel1[:, 0:128],
                            recips[2 * gp][:],
                            start=True,
                            stop=False,
                        )
                        nc.tensor.matmul(
                            bc[:],
                            sel1[:, 128:256],
                            recips[2 * gp + 1][:],
                            start=False,
                            stop=True,
                        )
                        nc.vector.tensor_mul(YT[i][:, tsl], YT[i][:, tsl], bc[:])

            pattn.close()

            # ---------------- Phase 3: output projection ----------------
            with ExitStack() as ph3:
                po_ = ph3.enter_context(
                    tc.tile_pool(name="po", bufs=4, space="PSUM")
                )
                pout = ph3.enter_context(tc.tile_pool(name="pout", bufs=4))
                for co in range(4):
                    csl = slice(512 * co, 512 * (co + 1))
                    for tt in range(16):
                        op = po_.tile([128, 512], FP32, tag="o", name="op")
                        for k2 in range(4):
                            nc.tensor.matmul(
                                op[:],
                                _r(YT[k2][:, 128 * tt : 128 * (tt + 1)]),
                                _r(wo_sb[k2][:, csl]),
                                start=(k2 == 0),
                                stop=(k2 == 3),
                            )
                        ot = pout.tile([128, 512], FP32, tag="ot", name="ot")
                        nc.scalar.copy(ot[:], op[:])
                        nc.sync.dma_start(
                            partial[128 * tt : 128 * (tt + 1), csl], ot[:]
                        )

        # ---------------- Phase 4: reduce-scatter + output ----------------
        nc.gpsimd.collective_compute(
            "ReduceScatter",
            mybir.AluOpType.add,
            replica_groups=GROUPS,
            ins=[partial],
            outs=[rs_out],
        )
        nc.sync.dma_start(out_ext, rs_out)


_NC_CACHE = None


def _get_nc():
    global _NC_CACHE
    if _NC_CACHE is None:
        _NC_CACHE = _build_program()
    return _NC_CACHE


def _consts():
    c = np.zeros((128, 384), np.float32)
    c[:128, :128] = np.eye(128, dtype=np.float32)
    c[0, 128:192] = 1.0
    c[0, 320:384] = 1.0
    c[:, 130] = 1.0  # ones column for v_aug (sel1 col 2 is already 1)
    return c


def make_in_maps(x, Wq, bq, Wk, bk, Wv, bv, Wo, bo):
    in_maps = []
    for r in range(N_CORES):
        b, rank = divmod(r, 4)
        qs = slice(512 * rank, 512 * (rank + 1))
        ks = slice(128 * rank, 128 * (rank + 1))
        wqkv = np.concatenate(
            [Wq[:, qs], Wk[:, ks], Wv[:, ks]], axis=1
        ).astype(np.float32)
        bcat = np.concatenate([bq[qs], bk[ks], bv[ks]]).astype(np.float32)
        in_maps.append(
            {
                "xT": np.ascontiguousarray(x[b].T).astype(np.float32),
                "wqkv": np.ascontiguousarray(wqkv),
                "bqkv": np.ascontiguousarray(bcat.reshape(6, 128).T),
                "wo": np.ascontiguousarray(Wo[qs, :]).astype(np.float32),
                "consts": _consts(),
            }
        )
    return in_maps


def kernel(x, Wq, bq, Wk, bk, Wv, bv, Wo, bo, _trace=False):
    x = np.asarray(x)
    nc = _get_nc()
    in_maps = make_in_maps(
        np.asarray(x, np.float32),
        np.asarray(Wq, np.float32),
        np.asarray(bq, np.float32),
        np.asarray(Wk, np.float32),
        np.asarray(bk, np.float32),
        np.asarray(Wv, np.float32),
        np.asarray(bv, np.float32),
        np.asarray(Wo, np.float32),
        np.asarray(bo, np.float32),
    )
    import time as _time

    t0 = _time.perf_counter()
    res = run_bass_kernel_spmd(nc, in_maps, list(range(N_CORES)), trace=_trace)
    kernel.last_spmd_wall_ns = int((_time.perf_counter() - t0) * 1e9)
    bo32 = np.asarray(bo, np.float32)
    out = np.empty((2, T, C), np.float32)
    for r in range(N_CORES):
        b, rank = divmod(r, 4)
        out[b, 512 * rank : 512 * (rank + 1), :] = (
            res.results[r]["out"] + bo32
        )
    kernel.last_exec_time_ns = res.exec_time_ns
    return out

